# revision 4
# baseline (speedup 1.0000x reference)
"""Trainium2 Bass kernel for nn_DenseEdgeModel — fp8 DoubleRow v2.

See kernel_fp8.py for the math/quantization scheme (identical):
  xx=(16d)^2 fp8 <=80; W1*4 fp8 -> psum=1024*z1; h0=relu(ps+1024*b1) fp8 <=113
  W2*4 fp8 -> psum=4096*z2; h1=relu(ps+4096*b2) bf16; w3=w3/4096 bf16; l3 bf16
  quad-packed via tile_position.

v2 vs v1: no GPSIMD (its per-instruction overhead dominated v1); xx is 4 ACT
Square instrs; l3 PSUM is zero-seeded twice per kernel and b3 is added in the
drain (saves the per-quad K=1 seed matmul); h1 lives in one [128,1024] bf16
tile; stage-2 uses the 3-deep software-pipeline skew (emit l3(i-3), l2(i-2),
l1(i-1), xx(i)) so every engine's program order interleaves groups whose
dependencies are already in flight.

Env knobs: KERNEL_H1 = dve (default; both h1-relu halves on DVE) | split
(ACT half 0 / DVE half 1). KERNEL_DRAIN = dve (default) | dma (DMA straight
from PSUM; skips the SBUF bounce but loses the drain-side b3 add -> uses the
per-quad seed matmul instead).
"""

import os

import numpy as np

import concourse.bass as bass
import concourse.tile as tile
from concourse import bacc, mybir
from concourse.bass_utils import run_bass_kernel_spmd

B, C, H, W = 2, 256, 32, 32
PK, CK = 256, 256
N_CORES = 8
CORES_PER_BATCH = N_CORES // B          # 4
P_SHARD = PK // CORES_PER_BATCH         # 64 primary indices per core
NJ = P_SHARD + CK                       # 320 gathered pixel columns per core
PAIR = 2                                # primary columns per stage-2 group
NF = PAIR * CK                          # 512 = stage-2 matmul free dim
QUAD = 4                                # groups sharing one l3 PSUM bank
NG = P_SHARD // PAIR                    # 32 groups

F32 = mybir.dt.float32
BF16 = mybir.dt.bfloat16
F8 = mybir.dt.float8e4
AF = mybir.ActivationFunctionType
OP = mybir.AluOpType
DR = mybir.MatmulPerfMode.DoubleRow

SXX = 16.0         # d scaling; xx stored = (SXX*d)^2 = 256*d^2
A1 = 4.0           # W1 scale -> psum1 = 1024*z1
A2 = 4.0           # W2 scale -> psum2 = 4096*z2
SH1 = 4096.0       # h1 stored scale; w3 folded by 1/SH1 on host

H1_MODE = os.environ.get("KERNEL_H1", "dve")
DRAIN = os.environ.get("KERNEL_DRAIN", "dve")


def _build_nc(reps=1):
    nc = bacc.Bacc("TRN2", target_bir_lowering=False, debug=False)

    xgT = nc.dram_tensor("xgT", [C, NJ], BF16, kind="ExternalInput")
    pre_wT = nc.dram_tensor("pre_wT", [3, C, C], BF16, kind="ExternalInput")
    pre_b = nc.dram_tensor("pre_b", [3, C], F32, kind="ExternalInput")
    w1p = nc.dram_tensor("w1p", [2, 128, 2 * 128], F8, kind="ExternalInput")
    w2p = nc.dram_tensor("w2p", [2, 128, 2 * 128], F8, kind="ExternalInput")
    b1s = nc.dram_tensor("b1s", [C], F32, kind="ExternalInput")
    b2s = nc.dram_tensor("b2s", [C], F32, kind="ExternalInput")
    w3T = nc.dram_tensor("w3T", [C, 2], BF16, kind="ExternalInput")
    b3row_d = nc.dram_tensor("b3row", [1, 128], BF16, kind="ExternalInput")
    ones_d = nc.dram_tensor("ones_row", [1, NF], BF16, kind="ExternalInput")
    zrow_d = nc.dram_tensor("zrow", [1, 128], BF16, kind="ExternalInput")
    b3col_d = nc.dram_tensor("b3col", [128, 1], F32, kind="ExternalInput")
    deadstore = os.environ.get("KERNEL_DEADSTORE", "") == "1"
    oshape = ([2, P_SHARD, CK] if reps == 1 or deadstore
              else [reps, 2, P_SHARD, CK])
    out = nc.dram_tensor("out", oshape, F32, kind="ExternalOutput")

    with tile.TileContext(nc) as tc:
        for r in range(reps):
            out_r = out.ap() if reps == 1 or deadstore else out[r]
            _kernel_body(nc, tc, xgT, pre_wT, pre_b, w1p, w2p, b1s, b2s, w3T,
                         b3row_d, ones_d, zrow_d, b3col_d, out_r)
    nc.compile()
    return nc


def _kernel_body(nc, tc, xgT, pre_wT, pre_b, w1p, w2p, b1s, b2s, w3T,
                 b3row_d, ones_d, zrow_d, b3col_d, out):
    with (
        tc.tile_pool(name="consts", bufs=1) as consts,
        tc.tile_pool(name="feats", bufs=1) as feats,
        tc.tile_pool(name="work", bufs=3) as work,
        tc.tile_pool(name="psum", bufs=1, space="PSUM") as psum,
    ):
        def col(ap_1d):  # [n] -> [n,1]
            return ap_1d.rearrange("(n one) -> n one", one=1)

        # ---- weights / biases to SBUF ----
        wpre = [[consts.tile([128, C], BF16, name=f"wpre_{i}_{cc}")
                 for cc in range(2)] for i in range(3)]
        bpre = [[consts.tile([128, 1], F32, name=f"bpre_{i}_{oc}")
                 for oc in range(2)] for i in range(3)]
        for i in range(3):
            for cc in range(2):
                nc.sync.dma_start(out=wpre[i][cc][:],
                                  in_=pre_wT[i, cc * 128:(cc + 1) * 128, :])
            for oc in range(2):
                nc.sync.dma_start(out=bpre[i][oc][:],
                                  in_=col(pre_b[i, oc * 128:(oc + 1) * 128]))
        w1t = [consts.tile([128, 2 * 128], F8, name=f"w1t_{oc}")
               for oc in range(2)]
        w2t = [consts.tile([128, 2 * 128], F8, name=f"w2t_{oc}")
               for oc in range(2)]
        b1c = [consts.tile([128, 1], F32, name=f"b1c_{oc}") for oc in range(2)]
        b2c = [consts.tile([128, 1], F32, name=f"b2c_{oc}") for oc in range(2)]
        for oc in range(2):
            nc.sync.dma_start(out=w1t[oc][:], in_=w1p[oc])
            nc.sync.dma_start(out=w2t[oc][:], in_=w2p[oc])
            nc.sync.dma_start(out=b1c[oc][:],
                              in_=col(b1s[oc * 128:(oc + 1) * 128]))
            nc.sync.dma_start(out=b2c[oc][:],
                              in_=col(b2s[oc * 128:(oc + 1) * 128]))
        w3 = [consts.tile([128, 2], BF16, name=f"w3_{cc}") for cc in range(2)]
        for cc in range(2):
            nc.sync.dma_start(out=w3[cc][:],
                              in_=w3T[cc * 128:(cc + 1) * 128, :])
        ones_row = consts.tile([1, NF], BF16, name="ones_row")
        nc.sync.dma_start(out=ones_row[:], in_=ones_d.ap())
        if DRAIN in ("dma", "seeddve"):
            b3row = consts.tile([1, 128], BF16, name="b3row")
            nc.sync.dma_start(out=b3row[:], in_=b3row_d.ap())
        else:
            zrow = consts.tile([1, 128], BF16, name="zrow")
            nc.sync.dma_start(out=zrow[:], in_=zrow_d.ap())
            b3c = consts.tile([128, 1], F32, name="b3c")
            nc.sync.dma_start(out=b3c[:], in_=b3col_d.ap())

        # ---- stage 1: pre-MLP (bf16) ----
        cur = [feats.tile([128, NJ], BF16, name=f"xg_{cc}") for cc in range(2)]
        for cc in range(2):
            nc.sync.dma_start(out=cur[cc][:],
                              in_=xgT[cc * 128:(cc + 1) * 128, :])
        for i in range(3):
            nxt = [feats.tile([128, NJ], BF16 if i < 2 else F32,
                              name=f"feat{i}_{oc}") for oc in range(2)]
            for oc in range(2):
                ps = psum.tile([128, NJ], F32, name=f"ps_s1_{i}_{oc}",
                               tag="ps_a", bufs=3)
                nc.tensor.matmul(
                    out=ps[:],
                    lhsT=wpre[i][0][:, oc * 128:(oc + 1) * 128],
                    rhs=cur[0][:], start=True, stop=False)
                nc.tensor.matmul(
                    out=ps[:],
                    lhsT=wpre[i][1][:, oc * 128:(oc + 1) * 128],
                    rhs=cur[1][:], start=False, stop=True)
                nc.scalar.activation(
                    out=nxt[oc][:], in_=ps[:],
                    func=AF.Relu if i < 2 else AF.Identity,
                    bias=bpre[i][oc][:, 0:1], scale=1.0)
            cur = nxt
        F = cur  # fp32 [128, NJ] x2

        fps = [feats.tile([128, P_SHARD], F32, name=f"fps_{cc}")
               for cc in range(2)]
        for cc in range(2):
            nc.scalar.activation(out=fps[cc][:], in_=F[cc][:, 0:P_SHARD],
                                 func=AF.Copy, scale=SXX)

        def two(ap):
            return ap.rearrange("k (two n) -> k two n", two=2)

        # ---- stage 2: 3-deep skewed pipeline over 32 groups ----
        xx_t, h0_t, h1_t, ps3_t = {}, {}, {}, {}

        def stage_xx(g):
            xx = work.tile([128, 2 * NF], F8, name="xx", tag=f"xx{g % 4}",
                           bufs=2)
            for s in range(PAIR):
                p = g * PAIR + s
                for cc in range(2):
                    nc.scalar.activation(
                        out=xx[:, cc * NF + s * CK:cc * NF + (s + 1) * CK],
                        in_=F[cc][:, P_SHARD:NJ], func=AF.Square,
                        bias=fps[cc][:, p:p + 1], scale=-SXX)
            xx_t[g] = xx

        def l1(g):
            h0_t[g] = work.tile([128, 2 * NF], F8, name="h0", tag=f"h0{g % 4}",
                                bufs=2)
            for oc in range(2):
                ps = psum.tile([128, NF], F32, name="ps1", tag="ps_a", bufs=3)
                nc.tensor.matmul(out=ps[:], lhsT=two(w1t[oc][:]),
                                 rhs=two(xx_t[g][:]), start=True, stop=True,
                                 perf_mode=DR)
                nc.vector.tensor_scalar(
                    out=h0_t[g][:, oc * NF:(oc + 1) * NF], in0=ps[:],
                    scalar1=b1c[oc][:, 0:1], scalar2=0.0,
                    op0=OP.add, op1=OP.max)
            xx_t.pop(g)

        def l2(g):
            h1_t[g] = work.tile([128, 2 * NF], BF16, name="h1",
                                tag=f"h1{g % 4}", bufs=2)
            for oc in range(2):
                ps = psum.tile([128, NF], F32, name="ps2", tag="ps_b", bufs=3)
                nc.tensor.matmul(out=ps[:], lhsT=two(w2t[oc][:]),
                                 rhs=two(h0_t[g][:]), start=True, stop=True,
                                 perf_mode=DR)
                if H1_MODE == "split" and oc == 0:
                    nc.scalar.activation(
                        out=h1_t[g][:, 0:NF], in_=ps[:], func=AF.Relu,
                        bias=b2c[0][:, 0:1], scale=1.0)
                else:
                    nc.vector.tensor_scalar(
                        out=h1_t[g][:, oc * NF:(oc + 1) * NF], in0=ps[:],
                        scalar1=b2c[oc][:, 0:1], scalar2=0.0,
                        op0=OP.add, op1=OP.max)
            h0_t.pop(g)

        def l3(g):
            hcur = h1_t.pop(g)
            k = g % QUAD
            if k == 0:
                ps3 = psum.tile([128, NF], F32, name="ps3", tag="ps3", bufs=2)
                if DRAIN in ("dma", "seeddve"):
                    nc.tensor.matmul(out=ps3[:], lhsT=b3row[:],
                                     rhs=ones_row[:], start=True, stop=True)
                else:
                    # zero-seed defines all 128 bank rows (the DVE drain reads
                    # the full bank) and, as the quad's first writer without
                    # skip_group_check, orders this quad after the previous
                    # drain of the recycled bank
                    nc.tensor.matmul(out=ps3[:], lhsT=zrow[:],
                                     rhs=ones_row[:], start=True, stop=True)
                ps3_t[g // QUAD] = ps3
            ps3 = ps3_t[g // QUAD]
            first_start = False
            nc.tensor.matmul(out=ps3[32 * k:32 * k + 2, :], lhsT=w3[0][:],
                             rhs=hcur[:, 0:NF], tile_position=(0, 32 * k),
                             start=first_start, stop=True,
                             skip_group_check=True)
            nc.tensor.matmul(out=ps3[32 * k:32 * k + 2, :], lhsT=w3[1][:],
                             rhs=hcur[:, NF:2 * NF], tile_position=(0, 32 * k),
                             start=False, stop=True, skip_group_check=True)
            if k == QUAD - 1:
                ps3_t.pop(g // QUAD)
                qb = (g // QUAD) * QUAD * PAIR
                if DRAIN == "dma":
                    for kk in range(QUAD):
                        nc.sync.dma_start(
                            out=out[:, qb + kk * PAIR:qb + (kk + 1) * PAIR, :],
                            in_=ps3[32 * kk:32 * kk + 2, :].rearrange(
                                "j (s q) -> j s q", s=PAIR))
                    return
                ob = work.tile([128, NF], F32, name="ob", tag="ob", bufs=2)
                if DRAIN == "seeddve":
                    nc.vector.tensor_copy(out=ob[:], in_=ps3[:])
                else:
                    # drain on ACT: DVE (h0+h1 relus) is the busiest engine;
                    # ACT only carries the xx squares
                    nc.scalar.activation(out=ob[:], in_=ps3[:],
                                         func=AF.Identity,
                                         bias=b3c[:, 0:1], scale=1.0)
                for kk in range(QUAD):
                    nc.sync.dma_start(
                        out=out[:, qb + kk * PAIR:qb + (kk + 1) * PAIR, :],
                        in_=ob[32 * kk:32 * kk + 2, :].rearrange(
                            "j (s q) -> j s q", s=PAIR))

        for i in range(NG + 3):
            if i >= 3:
                l3(i - 3)
            if 2 <= i < NG + 2:
                l2(i - 2)
            if 1 <= i < NG + 1:
                l1(i - 1)
            if i < NG:
                stage_xx(i)


_NC_CACHE = {}


def _get_nc():
    if "nc" not in _NC_CACHE:
        _NC_CACHE["nc"] = _build_nc()
    return _NC_CACHE["nc"]


def _pack_dr(w, alpha):
    """[O,C] weight -> DoubleRow lhsT [oc, k, (i,m)] fp8: w*alpha at
    [oc*128+m, i*128+k]."""
    import ml_dtypes
    ws = np.asarray(w, np.float64) * alpha
    assert np.abs(ws).max() <= 240.0
    wt = ws.T.reshape(2, 128, 2, 128)            # [i, k, oc, m]
    return np.ascontiguousarray(
        wt.transpose(2, 1, 0, 3).reshape(2, 128, 256)
    ).astype(ml_dtypes.float8_e4m3)


def _shard_inputs(x, primary_indices, compare_indices, pre_w, pre_b,
                  post_w, post_b, post_out_w, post_out_b):
    import ml_dtypes
    BF = ml_dtypes.bfloat16
    x = np.asarray(x, dtype=np.float32)
    x_rows = np.ascontiguousarray(x.transpose(0, 2, 3, 1)).reshape(B * H * W, C)
    pre_wT = np.ascontiguousarray(
        np.asarray(pre_w, dtype=np.float32).transpose(0, 2, 1)).astype(BF)
    pre_b = np.ascontiguousarray(np.asarray(pre_b, dtype=np.float32))
    post_w = np.asarray(post_w, dtype=np.float32)
    post_b = np.asarray(post_b, dtype=np.float32)
    w1p = _pack_dr(post_w[0], A1)
    w2p = _pack_dr(post_w[1], A2)
    b1s = np.ascontiguousarray(SXX * SXX * A1 * post_b[0]).astype(np.float32)
    b2s = np.ascontiguousarray(SH1 * post_b[1]).astype(np.float32)
    w3T = np.ascontiguousarray(
        np.asarray(post_out_w, dtype=np.float64).T / SH1).astype(BF)
    b3 = np.asarray(post_out_b, dtype=np.float32)
    b3row = np.zeros((1, 128), dtype=np.float32)
    b3col = np.zeros((128, 1), dtype=np.float32)
    for k in range(QUAD):
        b3row[0, 32 * k:32 * k + 2] = b3
        b3col[32 * k:32 * k + 2, 0] = b3
    b3row = b3row.astype(BF)
    ones_row = np.ones((1, NF), dtype=np.float32).astype(BF)
    zrow = np.zeros((1, 128), dtype=np.float32).astype(BF)
    primary_indices = np.asarray(primary_indices)
    compare_indices = np.asarray(compare_indices)

    in_maps = []
    for core in range(N_CORES):
        b = core // CORES_PER_BATCH
        ps = (core % CORES_PER_BATCH) * P_SHARD
        rows = np.concatenate([
            primary_indices[b, ps:ps + P_SHARD].astype(np.int64),
            compare_indices[b].astype(np.int64),
        ])
        xg_T = np.ascontiguousarray(x_rows[rows].T).astype(BF)  # [C, NJ]
        in_maps.append({
            "xgT": xg_T,
            "pre_wT": pre_wT,
            "pre_b": pre_b,
            "w1p": w1p,
            "w2p": w2p,
            "b1s": b1s,
            "b2s": b2s,
            "w3T": w3T,
            "b3row": b3row,
            "ones_row": ones_row,
            "zrow": zrow,
            "b3col": b3col,
        })
    return in_maps


def _unshard_output(results):
    out = np.empty((B, 2, PK, CK), dtype=np.float32)
    for core in range(N_CORES):
        b = core // CORES_PER_BATCH
        ps = (core % CORES_PER_BATCH) * P_SHARD
        out[b, :, ps:ps + P_SHARD, :] = results[core]["out"]
    return out


def kernel(x, primary_indices, compare_indices, pre_w, pre_b,
           post_w, post_b, post_out_w, post_out_b):
    in_maps = _shard_inputs(x, primary_indices, compare_indices, pre_w, pre_b,
                            post_w, post_b, post_out_w, post_out_b)
    nc = _get_nc()
    res = run_bass_kernel_spmd(nc, in_maps, core_ids=list(range(N_CORES)))
    return _unshard_output(res.results)


# revision 8
# speedup vs baseline: 1.1966x; 1.1966x over previous
"""Trainium2 Bass kernel for nn_DenseEdgeModel — fp8 DoubleRow v2.

See kernel_fp8.py for the math/quantization scheme (identical):
  xx=(16d)^2 fp8 <=80; W1*4 fp8 -> psum=1024*z1; h0=relu(ps+1024*b1) fp8 <=113
  W2*4 fp8 -> psum=4096*z2; h1=relu(ps+4096*b2) bf16; w3=w3/4096 bf16; l3 bf16
  quad-packed via tile_position.

v2 vs v1: no GPSIMD (its per-instruction overhead dominated v1); xx is 4 ACT
Square instrs; l3 PSUM is zero-seeded twice per kernel and b3 is added in the
drain (saves the per-quad K=1 seed matmul); h1 lives in one [128,1024] bf16
tile; stage-2 uses the 3-deep software-pipeline skew (emit l3(i-3), l2(i-2),
l1(i-1), xx(i)) so every engine's program order interleaves groups whose
dependencies are already in flight.

Env knobs: KERNEL_H1 = dve (default; both h1-relu halves on DVE) | split
(ACT half 0 / DVE half 1). KERNEL_DRAIN = dve (default) | dma (DMA straight
from PSUM; skips the SBUF bounce but loses the drain-side b3 add -> uses the
per-quad seed matmul instead).
"""

import os

import numpy as np

import concourse.bass as bass
import concourse.tile as tile
from concourse import bacc, mybir
from concourse.bass_utils import run_bass_kernel_spmd

B, C, H, W = 2, 256, 32, 32
PK, CK = 256, 256
N_CORES = 8
CORES_PER_BATCH = N_CORES // B          # 4
P_SHARD = PK // CORES_PER_BATCH         # 64 primary indices per core
NJ = P_SHARD + CK                       # 320 gathered pixel columns per core
PAIR = 2                                # primary columns per stage-2 group
NF = PAIR * CK                          # 512 = stage-2 matmul free dim
QUAD = 4                                # groups sharing one l3 PSUM bank
NG = P_SHARD // PAIR                    # 32 groups

F32 = mybir.dt.float32
BF16 = mybir.dt.bfloat16
F8 = mybir.dt.float8e4
AF = mybir.ActivationFunctionType
OP = mybir.AluOpType
DR = mybir.MatmulPerfMode.DoubleRow

SXX = 16.0         # d scaling; xx stored = (SXX*d)^2 = 256*d^2
A1 = 4.0           # W1 scale -> psum1 = 1024*z1
A2 = 4.0           # W2 scale -> psum2 = 4096*z2
SH1 = 4096.0       # h1 stored scale; w3 folded by 1/SH1 on host

H1_MODE = os.environ.get("KERNEL_H1", "dve")
DRAIN = os.environ.get("KERNEL_DRAIN", "dve")


def _build_nc(reps=1):
    nc = bacc.Bacc("TRN2", target_bir_lowering=False, debug=False)

    xgT = nc.dram_tensor("xgT5", [C, NJ], BF16, kind="ExternalInput")
    pre_wT = nc.dram_tensor("pre_wT", [3, C, C], BF16, kind="ExternalInput")
    pre_b = nc.dram_tensor("pre_b", [3, C], F32, kind="ExternalInput")
    w1p = nc.dram_tensor("w1p", [2, 128, 2 * 128], F8, kind="ExternalInput")
    w2p = nc.dram_tensor("w2p", [2, 128, 2 * 128], F8, kind="ExternalInput")
    b1s = nc.dram_tensor("b1s", [C], F32, kind="ExternalInput")
    b2s = nc.dram_tensor("b2s", [C], F32, kind="ExternalInput")
    w3T = nc.dram_tensor("w3T", [C, 2], BF16, kind="ExternalInput")
    b3row_d = nc.dram_tensor("b3row", [1, 128], BF16, kind="ExternalInput")
    ones_d = nc.dram_tensor("ones_row", [1, NF], BF16, kind="ExternalInput")
    zrow_d = nc.dram_tensor("zrow", [1, 128], BF16, kind="ExternalInput")
    b3col_d = nc.dram_tensor("b3col", [128, 1], F32, kind="ExternalInput")
    deadstore = os.environ.get("KERNEL_DEADSTORE", "") == "1"
    oshape = ([2, P_SHARD, CK] if reps == 1 or deadstore
              else [reps, 2, P_SHARD, CK])
    out = nc.dram_tensor("out", oshape, F32, kind="ExternalOutput")

    with tile.TileContext(nc) as tc:
        for r in range(reps):
            out_r = out.ap() if reps == 1 or deadstore else out[r]
            _kernel_body(nc, tc, xgT, pre_wT, pre_b, w1p, w2p, b1s, b2s, w3T,
                         b3row_d, ones_d, zrow_d, b3col_d, out_r)
    nc.compile()
    return nc


def _kernel_body(nc, tc, xgT, pre_wT, pre_b, w1p, w2p, b1s, b2s, w3T,
                 b3row_d, ones_d, zrow_d, b3col_d, out):
    with (
        tc.tile_pool(name="consts", bufs=1) as consts,
        tc.tile_pool(name="feats", bufs=1) as feats,
        tc.tile_pool(name="work", bufs=3) as work,
        tc.tile_pool(name="psum", bufs=1, space="PSUM") as psum,
    ):
        def col(ap_1d):  # [n] -> [n,1]
            return ap_1d.rearrange("(n one) -> n one", one=1)

        # ---- inputs/weights to SBUF; emission order = criticality ----
        # stage-1 layer 0 only needs xgT + wpre[0] + bpre[0]; emit those
        # DMAs first so the first matmul issues after ~166KB, not the full
        # ~650KB constant load (shorter PE-idle prologue, less HAM cold
        # throttle); the rest streams in under stage-1/stage-2 compute.
        cur = [feats.tile([128, NJ], BF16, name=f"xg_{cc}") for cc in range(2)]
        for cc in range(2):
            nc.sync.dma_start(out=cur[cc][:],
                              in_=xgT[cc * 128:(cc + 1) * 128, :])
        wpre = [[consts.tile([128, C], BF16, name=f"wpre_{i}_{cc}")
                 for cc in range(2)] for i in range(3)]
        bpre = [[consts.tile([128, 1], F32, name=f"bpre_{i}_{oc}")
                 for oc in range(2)] for i in range(3)]
        for i in range(3):
            for cc in range(2):
                nc.sync.dma_start(out=wpre[i][cc][:],
                                  in_=pre_wT[i, cc * 128:(cc + 1) * 128, :])
            for oc in range(2):
                nc.sync.dma_start(out=bpre[i][oc][:],
                                  in_=col(pre_b[i, oc * 128:(oc + 1) * 128]))
        w1t = [consts.tile([128, 2 * 128], F8, name=f"w1t_{oc}")
               for oc in range(2)]
        w2t = [consts.tile([128, 2 * 128], F8, name=f"w2t_{oc}")
               for oc in range(2)]
        b1c = [consts.tile([128, 1], F32, name=f"b1c_{oc}") for oc in range(2)]
        b2c = [consts.tile([128, 1], F32, name=f"b2c_{oc}") for oc in range(2)]
        for oc in range(2):
            nc.sync.dma_start(out=w1t[oc][:], in_=w1p[oc])
            nc.sync.dma_start(out=w2t[oc][:], in_=w2p[oc])
            nc.sync.dma_start(out=b1c[oc][:],
                              in_=col(b1s[oc * 128:(oc + 1) * 128]))
            nc.sync.dma_start(out=b2c[oc][:],
                              in_=col(b2s[oc * 128:(oc + 1) * 128]))
        w3 = [consts.tile([128, 2], BF16, name=f"w3_{cc}") for cc in range(2)]
        for cc in range(2):
            nc.sync.dma_start(out=w3[cc][:],
                              in_=w3T[cc * 128:(cc + 1) * 128, :])
        ones_row = consts.tile([1, NF], BF16, name="ones_row")
        nc.sync.dma_start(out=ones_row[:], in_=ones_d.ap())
        if DRAIN in ("dma", "seeddve"):
            b3row = consts.tile([1, 128], BF16, name="b3row")
            nc.sync.dma_start(out=b3row[:], in_=b3row_d.ap())
        else:
            zrow = consts.tile([1, 128], BF16, name="zrow")
            nc.sync.dma_start(out=zrow[:], in_=zrow_d.ap())
            b3c = consts.tile([128, 1], F32, name="b3c")
            nc.sync.dma_start(out=b3c[:], in_=b3col_d.ap())

        # ---- stage 1: pre-MLP (bf16) ----
        for i in range(3):
            nxt = [feats.tile([128, NJ], BF16 if i < 2 else F32,
                              name=f"feat{i}_{oc}") for oc in range(2)]
            for oc in range(2):
                ps = psum.tile([128, NJ], F32, name=f"ps_s1_{i}_{oc}",
                               tag="ps_a", bufs=3)
                nc.tensor.matmul(
                    out=ps[:],
                    lhsT=wpre[i][0][:, oc * 128:(oc + 1) * 128],
                    rhs=cur[0][:], start=True, stop=False)
                nc.tensor.matmul(
                    out=ps[:],
                    lhsT=wpre[i][1][:, oc * 128:(oc + 1) * 128],
                    rhs=cur[1][:], start=False, stop=True)
                nc.scalar.activation(
                    out=nxt[oc][:], in_=ps[:],
                    func=AF.Relu if i < 2 else AF.Identity,
                    bias=bpre[i][oc][:, 0:1], scale=1.0)
            cur = nxt
        F = cur  # fp32 [128, NJ] x2

        fps = [feats.tile([128, P_SHARD], F32, name=f"fps_{cc}")
               for cc in range(2)]
        for cc in range(2):
            nc.scalar.activation(out=fps[cc][:], in_=F[cc][:, 0:P_SHARD],
                                 func=AF.Copy, scale=SXX)

        def two(ap):
            return ap.rearrange("k (two n) -> k two n", two=2)

        # ---- stage 2: 3-deep skewed pipeline over 32 groups ----
        xx_t, h0_t, h1_t, ps3_t = {}, {}, {}, {}

        def stage_xx(g):
            xx = work.tile([128, 2 * NF], F8, name="xx", tag=f"xx{g % 4}",
                           bufs=2)
            for s in range(PAIR):
                p = g * PAIR + s
                for cc in range(2):
                    nc.scalar.activation(
                        out=xx[:, cc * NF + s * CK:cc * NF + (s + 1) * CK],
                        in_=F[cc][:, P_SHARD:NJ], func=AF.Square,
                        bias=fps[cc][:, p:p + 1], scale=-SXX)
            xx_t[g] = xx

        def l1(g):
            h0_t[g] = work.tile([128, 2 * NF], F8, name="h0", tag=f"h0{g % 4}",
                                bufs=2)
            for oc in range(2):
                ps = psum.tile([128, NF], F32, name="ps1", tag="ps_a", bufs=3)
                nc.tensor.matmul(out=ps[:], lhsT=two(w1t[oc][:]),
                                 rhs=two(xx_t[g][:]), start=True, stop=True,
                                 perf_mode=DR)
                nc.vector.tensor_scalar(
                    out=h0_t[g][:, oc * NF:(oc + 1) * NF], in0=ps[:],
                    scalar1=b1c[oc][:, 0:1], scalar2=0.0,
                    op0=OP.add, op1=OP.max)
            xx_t.pop(g)

        def l2(g):
            h1_t[g] = work.tile([128, 2 * NF], BF16, name="h1",
                                tag=f"h1{g % 4}", bufs=2)
            for oc in range(2):
                ps = psum.tile([128, NF], F32, name="ps2", tag="ps_b", bufs=3)
                nc.tensor.matmul(out=ps[:], lhsT=two(w2t[oc][:]),
                                 rhs=two(h0_t[g][:]), start=True, stop=True,
                                 perf_mode=DR)
                if H1_MODE == "split" and oc == 0:
                    nc.scalar.activation(
                        out=h1_t[g][:, 0:NF], in_=ps[:], func=AF.Relu,
                        bias=b2c[0][:, 0:1], scale=1.0)
                else:
                    nc.vector.tensor_scalar(
                        out=h1_t[g][:, oc * NF:(oc + 1) * NF], in0=ps[:],
                        scalar1=b2c[oc][:, 0:1], scalar2=0.0,
                        op0=OP.add, op1=OP.max)
            h0_t.pop(g)

        def l3(g):
            hcur = h1_t.pop(g)
            k = g % QUAD
            if k == 0:
                ps3 = psum.tile([128, NF], F32, name="ps3", tag="ps3", bufs=2)
                if DRAIN in ("dma", "seeddve"):
                    nc.tensor.matmul(out=ps3[:], lhsT=b3row[:],
                                     rhs=ones_row[:], start=True, stop=True)
                else:
                    # zero-seed defines all 128 bank rows (the DVE drain reads
                    # the full bank) and, as the quad's first writer without
                    # skip_group_check, orders this quad after the previous
                    # drain of the recycled bank
                    nc.tensor.matmul(out=ps3[:], lhsT=zrow[:],
                                     rhs=ones_row[:], start=True, stop=True)
                ps3_t[g // QUAD] = ps3
            ps3 = ps3_t[g // QUAD]
            first_start = False
            nc.tensor.matmul(out=ps3[32 * k:32 * k + 2, :], lhsT=w3[0][:],
                             rhs=hcur[:, 0:NF], tile_position=(0, 32 * k),
                             start=first_start, stop=True,
                             skip_group_check=True)
            nc.tensor.matmul(out=ps3[32 * k:32 * k + 2, :], lhsT=w3[1][:],
                             rhs=hcur[:, NF:2 * NF], tile_position=(0, 32 * k),
                             start=False, stop=True, skip_group_check=True)
            if k == QUAD - 1:
                ps3_t.pop(g // QUAD)
                qb = (g // QUAD) * QUAD * PAIR
                if DRAIN == "dma":
                    for kk in range(QUAD):
                        nc.sync.dma_start(
                            out=out[:, qb + kk * PAIR:qb + (kk + 1) * PAIR, :],
                            in_=ps3[32 * kk:32 * kk + 2, :].rearrange(
                                "j (s q) -> j s q", s=PAIR))
                    return
                ob = work.tile([128, NF], F32, name="ob", tag="ob", bufs=2)
                if DRAIN == "seeddve":
                    nc.vector.tensor_copy(out=ob[:], in_=ps3[:])
                    for kk in range(QUAD):
                        nc.sync.dma_start(
                            out=out[:, qb + kk * PAIR:qb + (kk + 1) * PAIR, :],
                            in_=ob[32 * kk:32 * kk + 2, :].rearrange(
                                "j (s q) -> j s q", s=PAIR))
                    return
                # drain on ACT: DVE (h0+h1 relus) is the busiest engine;
                # ACT only carries the xx squares
                nc.scalar.activation(out=ob[:], in_=ps3[:],
                                     func=AF.Identity,
                                     bias=b3c[:, 0:1], scale=1.0)
                # one DMA per output channel j: rows {32k+j} are a clean
                # stride-32 partition slice, so the whole quad drains in 2
                # DMAs instead of 4
                for j in range(PAIR):
                    nc.sync.dma_start(
                        out=out[j, qb:qb + QUAD * PAIR, :].rearrange(
                            "(k s) q -> k s q", k=QUAD),
                        in_=ob[j:j + 32 * (QUAD - 1) + 1:32, :].rearrange(
                            "k (s q) -> k s q", s=PAIR))

        for i in range(NG + 3):
            if i >= 3:
                l3(i - 3)
            if 2 <= i < NG + 2:
                l2(i - 2)
            if 1 <= i < NG + 1:
                l1(i - 1)
            if i < NG:
                stage_xx(i)


_NC_CACHE = {}


def _get_nc():
    if "nc" not in _NC_CACHE:
        _NC_CACHE["nc"] = _build_nc()
    return _NC_CACHE["nc"]


def _pack_dr(w, alpha):
    """[O,C] weight -> DoubleRow lhsT [oc, k, (i,m)] fp8: w*alpha at
    [oc*128+m, i*128+k]."""
    import ml_dtypes
    ws = np.asarray(w, np.float64) * alpha
    assert np.abs(ws).max() <= 240.0
    wt = ws.T.reshape(2, 128, 2, 128)            # [i, k, oc, m]
    return np.ascontiguousarray(
        wt.transpose(2, 1, 0, 3).reshape(2, 128, 256)
    ).astype(ml_dtypes.float8_e4m3)


def _shard_inputs(x, primary_indices, compare_indices, pre_w, pre_b,
                  post_w, post_b, post_out_w, post_out_b):
    import ml_dtypes
    BF = ml_dtypes.bfloat16
    x = np.asarray(x, dtype=np.float32)
    x_rows = np.ascontiguousarray(x.transpose(0, 2, 3, 1)).reshape(B * H * W, C)
    pre_wT = np.ascontiguousarray(
        np.asarray(pre_w, dtype=np.float32).transpose(0, 2, 1)).astype(BF)
    pre_b = np.ascontiguousarray(np.asarray(pre_b, dtype=np.float32))
    post_w = np.asarray(post_w, dtype=np.float32)
    post_b = np.asarray(post_b, dtype=np.float32)
    w1p = _pack_dr(post_w[0], A1)
    w2p = _pack_dr(post_w[1], A2)
    b1s = np.ascontiguousarray(SXX * SXX * A1 * post_b[0]).astype(np.float32)
    b2s = np.ascontiguousarray(SH1 * post_b[1]).astype(np.float32)
    w3T = np.ascontiguousarray(
        np.asarray(post_out_w, dtype=np.float64).T / SH1).astype(BF)
    b3 = np.asarray(post_out_b, dtype=np.float32)
    b3row = np.zeros((1, 128), dtype=np.float32)
    b3col = np.zeros((128, 1), dtype=np.float32)
    for k in range(QUAD):
        b3row[0, 32 * k:32 * k + 2] = b3
        b3col[32 * k:32 * k + 2, 0] = b3
    b3row = b3row.astype(BF)
    ones_row = np.ones((1, NF), dtype=np.float32).astype(BF)
    zrow = np.zeros((1, 128), dtype=np.float32).astype(BF)
    primary_indices = np.asarray(primary_indices)
    compare_indices = np.asarray(compare_indices)

    in_maps = []
    for core in range(N_CORES):
        b = core // CORES_PER_BATCH
        ps = (core % CORES_PER_BATCH) * P_SHARD
        rows = np.concatenate([
            primary_indices[b, ps:ps + P_SHARD].astype(np.int64),
            compare_indices[b].astype(np.int64),
        ])
        xg_T = np.ascontiguousarray(x_rows[rows].T).astype(BF)  # [C, NJ]
        in_maps.append({
            "xgT5": xg_T,
            "pre_wT": pre_wT,
            "pre_b": pre_b,
            "w1p": w1p,
            "w2p": w2p,
            "b1s": b1s,
            "b2s": b2s,
            "w3T": w3T,
            "b3row": b3row,
            "ones_row": ones_row,
            "zrow": zrow,
            "b3col": b3col,
        })
    return in_maps


def _unshard_output(results):
    out = np.empty((B, 2, PK, CK), dtype=np.float32)
    for core in range(N_CORES):
        b = core // CORES_PER_BATCH
        ps = (core % CORES_PER_BATCH) * P_SHARD
        out[b, :, ps:ps + P_SHARD, :] = results[core]["out"]
    return out


def kernel(x, primary_indices, compare_indices, pre_w, pre_b,
           post_w, post_b, post_out_w, post_out_b):
    in_maps = _shard_inputs(x, primary_indices, compare_indices, pre_w, pre_b,
                            post_w, post_b, post_out_w, post_out_b)
    nc = _get_nc()
    try:
        res = run_bass_kernel_spmd(nc, in_maps, core_ids=list(range(N_CORES)))
    except Exception:
        # transient device wedges (e.g. NRT_EXEC_UNIT_UNRECOVERABLE) clear on
        # an identical rerun
        res = run_bass_kernel_spmd(nc, in_maps, core_ids=list(range(N_CORES)))
    return _unshard_output(res.results)
